# revision 5
# baseline (speedup 1.0000x reference)
"""Trainium2 Bass kernel for nn_CustomLoss_188978561648.

loss = -(1/K) * sum_{k,i} num[k,i] / (var + rs[k,i] - num[k,i])
  rs  = zs @ X.T          [K, N]   (the dominant GEMM)
  num = zs * diag(X)      [K, N]

Sharding: tensor-parallel over the output columns i (rows of X).
Core c owns i in [c*512, (c+1)*512).

v2 design (vs the 32 us fp16 baseline):
- fp8e4 (e4m3) matmul operands + MatmulPerfMode.DoubleRow: halves both
  the HBM stream (2.3 MB/core) and the PE column count (16 matmuls,
  each contracting 256 rows). Measured host-side rel err ~1.6e-5 vs
  the 2e-2 gate (random quantization noise averages out over the
  4096-term contraction).
- X is diag-zeroed on the host, so the GEMM computes rs - num
  directly; +var is folded in as a rank-1 fp16 matmul (lhsT [1,64] of
  var, rhs [1,512] of ones) that runs at stream start. PSUM then holds
  the full denominator, so the epilogue is just
  rcp_approx_fast + one STT (num * rcp, fused free-axis accum).
- num = zs*diag shipped fp16; final cross-partition reduce on the PE
  against a (-1/K) ones vector; [1,1] fp32 output on the scalar ring.
"""

import numpy as np

K = 64          # schedules (zs rows)
N = 4096        # channel dim
NCORES = 8
SHARD = N // NCORES            # 512 output columns per core
NCHUNKS = N // 128             # 32 contraction chunks of 128
NPAIRS = NCHUNKS // 2          # 16 DoubleRow chunk pairs
# xt stream granularity (pairs per DMA) and ring: the 2 MB xt stream is
# split across BOTH HWDGE rings (sync + scalar) so descriptor issue and
# transfer overlap; the slightly slower scalar ring also carries the
# small control tensors (zst/num).
PAIR_BLOCKS = (4, 4, 2, 2, 2, 2)
BLOCK_RING = ("sync", "scalar", "sync", "scalar", "sync", "sync")
XCOLS = NCHUNKS * SHARD        # 16384 packed xt cols per partition

_CACHE = {}


def _build():
    import concourse.bacc as bacc
    import concourse.tile as tile
    import concourse.mybir as mybir
    f32 = mybir.dt.float32
    f16 = mybir.dt.float16
    f8 = mybir.dt.float8e4

    nc = bacc.Bacc(
        "TRN2", target_bir_lowering=False, debug=False, num_devices=NCORES
    )

    varrow_d = nc.dram_tensor("varrow", [1, K + SHARD], f16, kind="ExternalInput")
    zst_d = nc.dram_tensor("zst", [128, NCHUNKS * K], f8, kind="ExternalInput")
    xt_d = nc.dram_tensor("xt", [128, XCOLS], f8, kind="ExternalInput")
    num_d = nc.dram_tensor("num", [K, SHARD], f16, kind="ExternalInput")
    out_d = nc.dram_tensor("out", [1, 1], f32, kind="ExternalOutput")

    with tile.TileContext(nc) as tc:
        with (
            tc.tile_pool(name="data", bufs=1) as dpool,
            tc.tile_pool(name="ep", bufs=1) as epool,
            tc.tile_pool(name="ps", bufs=1, space="PSUM") as pspool,
        ):
            ones_t = epool.tile([K, 1], f32, tag="ones")
            nc.vector.memset(ones_t[:], -1.0 / K)

            # -- stream: sync ring carries varrow + its xt blocks (+ the
            #    final out DMA); scalar ring carries zst + its xt blocks +
            #    num. Each ring is FIFO in program order. --
            varrow_t = dpool.tile([1, K + SHARD], f16, tag="varrow")
            nc.sync.dma_start(varrow_t[:], varrow_d[:])
            zst_t = dpool.tile([128, NCHUNKS, K], f8, tag="zst")
            nc.scalar.dma_start(zst_t[:], zst_d[:])
            xt_t = []
            off = 0
            for b, npair in enumerate(PAIR_BLOCKS):
                cols = npair * 2 * SHARD
                t = dpool.tile([128, npair * 2, SHARD], f8, tag=f"xt{b}")
                eng = nc.sync if BLOCK_RING[b] == "sync" else nc.scalar
                eng.dma_start(t[:], xt_d[:, off : off + cols])
                xt_t.append(t)
                off += cols
            num_t = epool.tile([K, SHARD], f16, tag="num")
            nc.scalar.dma_start(num_t[:], num_d[:])

            # -- PE: +var rank-1 matmul, then 16 fp8 DoubleRow pair matmuls --
            ps = pspool.tile([K, SHARD], f32, tag="ps")
            nc.tensor.matmul(
                ps[:],
                varrow_t[:, :K],
                varrow_t[:, K:],
                start=True,
                stop=False,
                skip_group_check=True,
            )
            j = 0
            for b, npair in enumerate(PAIR_BLOCKS):
                for jj in range(npair):
                    nc.tensor.matmul(
                        ps[:],
                        zst_t[:, 2 * j : 2 * j + 2, :],
                        xt_t[b][:, 2 * jj : 2 * jj + 2, :],
                        start=False,
                        stop=(j == NPAIRS - 1),
                        perf_mode=mybir.MatmulPerfMode.DoubleRow,
                        skip_group_check=True,
                    )
                    j += 1

            # -- epilogue: PSUM already holds den = var + rs - num --
            rcp_t = epool.tile([K, SHARD], f32, tag="rcp")
            scr_t = epool.tile([K, SHARD], f32, tag="scr")
            red_t = epool.tile([K, 1], f32, tag="red")
            nc.vector.reciprocal_approx_fast(rcp_t[:], ps[:])
            nc.vector.scalar_tensor_tensor(
                out=scr_t[:], in0=num_t[:], scalar=1.0, in1=rcp_t[:],
                op0=mybir.AluOpType.mult, op1=mybir.AluOpType.mult,
                accum_out=red_t[:],
            )
            # cross-partition reduce on PE: out = red.T @ (-1/K * ones)
            ps1 = pspool.tile([1, 1], f32, tag="ps1")
            nc.tensor.matmul(ps1[:], red_t[:], ones_t[:], start=True, stop=True)
            out_sb = epool.tile([1, 1], f32, tag="out_sb")
            nc.vector.tensor_copy(out_sb[:], ps1[:])
            nc.sync.dma_start(out_d[:], out_sb[:], single_packet=True)

    nc.compile()
    return nc


def _prep_inputs(zs, X, var_noise):
    """Host-side shard + layout packing (layout + dtype cast; the only
    math is diag extraction, the zs*diag elementwise product, and
    zeroing X's diagonal)."""
    import ml_dtypes

    f8 = ml_dtypes.float8_e4m3
    zs = np.ascontiguousarray(np.asarray(zs, dtype=np.float32))
    X = np.asarray(X, dtype=np.float32)
    var = np.float32(np.asarray(var_noise).reshape(()))

    diag = np.ascontiguousarray(np.diagonal(X)).astype(np.float32)
    Xz = X.copy()
    np.fill_diagonal(Xz, 0.0)

    # zst[p, m, k] = zs[k, 128m + p], replicated across cores
    zst = np.ascontiguousarray(
        zs.reshape(K, NCHUNKS, 128).transpose(2, 1, 0)
    ).astype(f8).reshape(128, NCHUNKS * K)

    varrow = np.empty((1, K + SHARD), dtype=np.float16)
    varrow[0, :K] = var
    varrow[0, K:] = 1.0

    in_maps = []
    for c in range(NCORES):
        sl = slice(c * SHARD, (c + 1) * SHARD)
        # xt[p, m, il] = Xz[c*512 + il, 128m + p]
        xt = np.ascontiguousarray(
            Xz[sl].reshape(SHARD, NCHUNKS, 128).transpose(2, 1, 0)
        ).astype(f8).reshape(128, XCOLS)
        num = (zs[:, sl] * diag[sl][None, :]).astype(np.float16)
        in_maps.append({"varrow": varrow, "zst": zst, "xt": xt, "num": num})
    return in_maps


def _run(in_maps, **run_kwargs):
    from concourse.bass_utils import run_bass_kernel_spmd

    if "nc" not in _CACHE:
        _CACHE["nc"] = _build()
    nc = _CACHE["nc"]
    return run_bass_kernel_spmd(
        nc, in_maps, core_ids=list(range(NCORES)), **run_kwargs
    )


def kernel(zs, X, var_noise):
    in_maps = _prep_inputs(zs, X, var_noise)
    res = None
    for attempt in range(3):
        try:
            res = _run(in_maps).results
            break
        except Exception:
            if attempt == 2:
                raise
            import time

            time.sleep(2)
    total = np.float32(0.0)
    for c in range(NCORES):
        total += res[c]["out"].astype(np.float32).sum(dtype=np.float32)
    return np.float32(total)


# revision 7
# speedup vs baseline: 1.0653x; 1.0653x over previous
"""Trainium2 Bass kernel for nn_CustomLoss_188978561648.

loss = -(1/K) * sum_{k,i} num[k,i] / (var + rs[k,i] - num[k,i])
  rs  = zs @ X.T          [K, N]   (the dominant GEMM)
  num = zs * diag(X)      [K, N]

Sharding: tensor-parallel over the output columns i (rows of X).
Core c owns i in [c*512, (c+1)*512).

v2 design (vs the 32 us fp16 baseline):
- fp8e4 (e4m3) matmul operands + MatmulPerfMode.DoubleRow: halves both
  the HBM stream (2.3 MB/core) and the PE column count (16 matmuls,
  each contracting 256 rows). Measured host-side rel err ~1.6e-5 vs
  the 2e-2 gate (random quantization noise averages out over the
  4096-term contraction).
- X is diag-zeroed on the host, so the GEMM computes rs - num
  directly; +var is folded in as a rank-1 fp16 matmul (lhsT [1,64] of
  var, rhs [1,512] of ones) that runs at stream start. PSUM then holds
  the full denominator, so the epilogue is just
  rcp_approx_fast + one STT (num * rcp, fused free-axis accum).
- num = zs*diag shipped fp16; final cross-partition reduce on the PE
  against a (-1/K) ones vector; [1,1] fp32 output on the scalar ring.
"""

import numpy as np

K = 64          # schedules (zs rows)
N = 4096        # channel dim
NCORES = 8
SHARD = N // NCORES            # 512 output columns per core
NCHUNKS = N // 128             # 32 contraction chunks of 128
NPAIRS = NCHUNKS // 2          # 16 DoubleRow chunk pairs
# xt stream granularity (pairs per DMA). The whole xt stream rides the
# sync HWDGE ring (measured: the two rings together sustain no more
# aggregate BW than sync alone — the ~400 GB/s cap is per-core HBM, not
# descriptor processing — and they interfere). The tiny varrow/num
# tensors go on the otherwise-idle scalar ring so the sync ring's
# ~600 ns/DMA descriptor-issue slots are all spent on the stream.
PAIR_BLOCKS = (4, 4, 4, 2, 1, 1)
BLOCK_RING = ("sync",) * 6
XCOLS = NCHUNKS * SHARD        # 16384 packed xt cols per partition

_CACHE = {}


def _build():
    import concourse.bacc as bacc
    import concourse.tile as tile
    import concourse.mybir as mybir
    f32 = mybir.dt.float32
    f16 = mybir.dt.float16
    f8 = mybir.dt.float8e4

    nc = bacc.Bacc(
        "TRN2", target_bir_lowering=False, debug=False, num_devices=NCORES
    )

    varrow_d = nc.dram_tensor("varrow", [1, K + SHARD], f16, kind="ExternalInput")
    zst_d = nc.dram_tensor("zst", [128, NCHUNKS * K], f8, kind="ExternalInput")
    xt_d = nc.dram_tensor("xt", [128, XCOLS], f8, kind="ExternalInput")
    num_d = nc.dram_tensor("num", [K, SHARD], f16, kind="ExternalInput")
    out_d = nc.dram_tensor("out", [1, 1], f32, kind="ExternalOutput")

    with tile.TileContext(nc) as tc:
        with (
            tc.tile_pool(name="data", bufs=1) as dpool,
            tc.tile_pool(name="ep", bufs=1) as epool,
            tc.tile_pool(name="ps", bufs=1, space="PSUM") as pspool,
        ):
            ones_t = epool.tile([K, 1], f32, tag="ones")
            nc.vector.memset(ones_t[:], -1.0 / K)

            # -- stream: sync ring carries varrow + its xt blocks (+ the
            #    final out DMA); scalar ring carries zst + its xt blocks +
            #    num. Each ring is FIFO in program order. --
            varrow_t = dpool.tile([1, K + SHARD], f16, tag="varrow")
            nc.scalar.dma_start(varrow_t[:], varrow_d[:])
            zst_t = dpool.tile([128, NCHUNKS, K], f8, tag="zst")
            nc.sync.dma_start(zst_t[:], zst_d[:])
            xt_t = []
            off = 0
            for b, npair in enumerate(PAIR_BLOCKS):
                cols = npair * 2 * SHARD
                t = dpool.tile([128, npair * 2, SHARD], f8, tag=f"xt{b}")
                eng = nc.sync if BLOCK_RING[b] == "sync" else nc.scalar
                eng.dma_start(t[:], xt_d[:, off : off + cols])
                xt_t.append(t)
                off += cols
            num_t = epool.tile([K, SHARD], f16, tag="num")
            nc.scalar.dma_start(num_t[:], num_d[:])

            # -- PE: +var rank-1 matmul, then 16 fp8 DoubleRow pair matmuls --
            ps = pspool.tile([K, SHARD], f32, tag="ps")
            nc.tensor.matmul(
                ps[:],
                varrow_t[:, :K],
                varrow_t[:, K:],
                start=True,
                stop=False,
                skip_group_check=True,
            )
            j = 0
            for b, npair in enumerate(PAIR_BLOCKS):
                for jj in range(npair):
                    nc.tensor.matmul(
                        ps[:],
                        zst_t[:, 2 * j : 2 * j + 2, :],
                        xt_t[b][:, 2 * jj : 2 * jj + 2, :],
                        start=False,
                        stop=(j == NPAIRS - 1),
                        perf_mode=mybir.MatmulPerfMode.DoubleRow,
                        skip_group_check=True,
                    )
                    j += 1

            # -- epilogue: PSUM already holds den = var + rs - num --
            rcp_t = epool.tile([K, SHARD], f32, tag="rcp")
            scr_t = epool.tile([K, SHARD], f32, tag="scr")
            red_t = epool.tile([K, 1], f32, tag="red")
            nc.vector.reciprocal_approx_fast(rcp_t[:], ps[:])
            nc.vector.scalar_tensor_tensor(
                out=scr_t[:], in0=num_t[:], scalar=1.0, in1=rcp_t[:],
                op0=mybir.AluOpType.mult, op1=mybir.AluOpType.mult,
                accum_out=red_t[:],
            )
            # cross-partition reduce on PE: out = red.T @ (-1/K * ones)
            ps1 = pspool.tile([1, 1], f32, tag="ps1")
            nc.tensor.matmul(ps1[:], red_t[:], ones_t[:], start=True, stop=True)
            out_sb = epool.tile([1, 1], f32, tag="out_sb")
            nc.vector.tensor_copy(out_sb[:], ps1[:])
            nc.sync.dma_start(out_d[:], out_sb[:], single_packet=True)

    nc.compile()
    return nc


def _prep_inputs(zs, X, var_noise):
    """Host-side shard + layout packing (layout + dtype cast; the only
    math is diag extraction, the zs*diag elementwise product, and
    zeroing X's diagonal)."""
    import ml_dtypes

    f8 = ml_dtypes.float8_e4m3
    zs = np.ascontiguousarray(np.asarray(zs, dtype=np.float32))
    X = np.asarray(X, dtype=np.float32)
    var = np.float32(np.asarray(var_noise).reshape(()))

    diag = np.ascontiguousarray(np.diagonal(X)).astype(np.float32)
    Xz = X.copy()
    np.fill_diagonal(Xz, 0.0)

    # zst[p, m, k] = zs[k, 128m + p], replicated across cores
    zst = np.ascontiguousarray(
        zs.reshape(K, NCHUNKS, 128).transpose(2, 1, 0)
    ).astype(f8).reshape(128, NCHUNKS * K)

    varrow = np.empty((1, K + SHARD), dtype=np.float16)
    varrow[0, :K] = var
    varrow[0, K:] = 1.0

    in_maps = []
    for c in range(NCORES):
        sl = slice(c * SHARD, (c + 1) * SHARD)
        # xt[p, m, il] = Xz[c*512 + il, 128m + p]
        xt = np.ascontiguousarray(
            Xz[sl].reshape(SHARD, NCHUNKS, 128).transpose(2, 1, 0)
        ).astype(f8).reshape(128, XCOLS)
        num = (zs[:, sl] * diag[sl][None, :]).astype(np.float16)
        in_maps.append({"varrow": varrow, "zst": zst, "xt": xt, "num": num})
    return in_maps


def _run(in_maps, **run_kwargs):
    from concourse.bass_utils import run_bass_kernel_spmd

    if "nc" not in _CACHE:
        _CACHE["nc"] = _build()
    nc = _CACHE["nc"]
    return run_bass_kernel_spmd(
        nc, in_maps, core_ids=list(range(NCORES)), **run_kwargs
    )


def kernel(zs, X, var_noise):
    in_maps = _prep_inputs(zs, X, var_noise)
    res = None
    for attempt in range(3):
        try:
            res = _run(in_maps).results
            break
        except Exception:
            if attempt == 2:
                raise
            import time

            time.sleep(2)
    total = np.float32(0.0)
    for c in range(NCORES):
        total += res[c]["out"].astype(np.float32).sum(dtype=np.float32)
    return np.float32(total)
